# revision 19
# baseline (speedup 1.0000x reference)
"""Trainium2 Bass kernel for AdditiveAttention (nn_AdditiveAttention_44564580663638).

Data-parallel over batch: B=8 -> one batch element per NeuronCore (8 cores).

Per-core pipeline (Q=256, K=1024, D=256, H=128, DV=128):
  1. TensorE: project  A^T = W_q @ q^T  [H,Q],  B^T = W_k @ k^T  [H,K]
  2. ScalarE (the bottleneck): per query q, one activation instruction computes
     feat = tanh(B^T + A^T[:,q])  [128h x 1024k]  (broadcast-add fused via bias)
  3. TensorE: scores accumulate via one-hot-expanded w_v weights:
     lhsT_q[h,c] = w_v[h] * (c==q)  => matmul writes w_v . feat into PSUM
     partition q; 128 accumulating matmuls build the [128q, 512k] score block.
     The -1e6 softmax mask folds in as one extra K=1 matmul (ones x mask_row).
  4. ScalarE: E = exp(scores) straight out of PSUM; VectorE: Z, 1/Z, P = E/Z
  5. TensorE: transpose P -> P^T chunks; final matmul P^T.T @ V -> out [256,128]
"""

import os
import sys

for _p in ("/opt/trn_rl_repo", "/root/.axon_site/_ro/trn_rl_repo"):
    if os.path.isdir(_p) and _p not in sys.path:
        sys.path.insert(0, _p)

import numpy as np
import ml_dtypes

import concourse.bass as bass
import concourse.bacc as bacc
import concourse.tile as tile
import concourse.mybir as mybir
from concourse.bass_utils import run_bass_kernel_spmd
from concourse.masks import make_identity

B, Q, K, DQ, DK, H, DV = 8, 256, 1024, 256, 256, 128, 128
P = 128
QB = Q // P      # query blocks of 128
KC = K // P      # key chunks of 128
DC = DQ // P     # contraction chunks for the projections
KH = K // 512    # psum half-banks per score row
FP32 = mybir.dt.float32
BF16 = mybir.dt.bfloat16
BF16_NP = ml_dtypes.bfloat16
NEG = -1e6

_NC = None
LAST_RESULT = None


def _build():
    nc = bacc.Bacc("TRN2", target_bir_lowering=False, debug=False)
    ACT = mybir.ActivationFunctionType

    qT_ext = nc.declare_dram_parameter("qT", [DQ, Q], BF16, isOutput=False)
    kT_ext = nc.declare_dram_parameter("kT", [DK, K], BF16, isOutput=False)
    v_ext = nc.declare_dram_parameter("v", [K, DV], BF16, isOutput=False)
    wqT_ext = nc.declare_dram_parameter("wqT", [DQ, H], BF16, isOutput=False)
    wkT_ext = nc.declare_dram_parameter("wkT", [DK, H], BF16, isOutput=False)
    wvdiag_ext = nc.declare_dram_parameter("wvdiag", [H, P, P], BF16, isOutput=False)
    mrow_ext = nc.declare_dram_parameter("mrow", [1, K], BF16, isOutput=False)
    out_ext = nc.declare_dram_parameter("out", [Q, DV], FP32, isOutput=True)

    with tile.TileContext(nc) as tc:
        with (
            tc.tile_pool(name="const", bufs=1) as constp,
            tc.tile_pool(name="sb", bufs=2) as sbp,
            tc.tile_pool(name="feat", bufs=3) as featp,
            tc.tile_pool(name="ps", bufs=2, space="PSUM") as psp,
            tc.tile_pool(name="ps2", bufs=2, space="PSUM") as psp2,
        ):
            # critical-path inputs first: kT/wk (for B^T), qT/wq (for A^T)
            kT_sb = constp.tile([P, DC, K], BF16)
            for c in range(DC):
                for s in range(2):
                    nc.sync.dma_start(
                        kT_sb[:, c, s * 512 : (s + 1) * 512],
                        kT_ext[c * P : (c + 1) * P, s * 512 : (s + 1) * 512],
                    )
            wk_sb = constp.tile([P, DC, H], BF16)
            nc.sync.dma_start(wk_sb[:], wkT_ext.rearrange("(c p) h -> p c h", p=P))
            qT_sb = constp.tile([P, DC, Q], BF16)
            for c in range(DC):
                nc.sync.dma_start(qT_sb[:, c, :], qT_ext[c * P : (c + 1) * P, :])
            wq_sb = constp.tile([P, DC, H], BF16)
            nc.sync.dma_start(wq_sb[:], wqT_ext.rearrange("(c p) h -> p c h", p=P))

            # projections (bf16 results: feed the DVE 4x-mode pre-adds)
            at_sb = constp.tile([H, Q], FP32)
            at_ps = psp.tile([H, Q], FP32, tag="sc")
            for c in range(DC):
                nc.tensor.matmul(
                    at_ps[:], wq_sb[:, c, :], qT_sb[:, c, :],
                    start=(c == 0), stop=(c == DC - 1),
                )
            nc.vector.tensor_copy(at_sb[:], at_ps[:])

            bt_sb = constp.tile([H, K], BF16)
            for kh in range(KH):
                bt_ps = psp.tile([H, 512], FP32, tag="sc")
                for c in range(DC):
                    nc.tensor.matmul(
                        bt_ps[:], wk_sb[:, c, :], kT_sb[:, c, kh * 512 : (kh + 1) * 512],
                        start=(c == 0), stop=(c == DC - 1),
                    )
                nc.vector.tensor_copy(bt_sb[:, kh * 512 : (kh + 1) * 512], bt_ps[:])

            # non-critical inputs after the projection chain
            mrow_sb = constp.tile([1, K], BF16)
            nc.sync.dma_start(mrow_sb[:], mrow_ext[:, :])
            v_sb = constp.tile([P, KC, DV], BF16)
            nc.sync.dma_start(v_sb[:], v_ext.rearrange("(c p) d -> p c d", p=P))
            ones_sb = constp.tile([1, P], BF16)
            nc.vector.memset(ones_sb[:], 1.0)
            ident = constp.tile([P, P], BF16)
            make_identity(nc, ident[:])
            # one-hot expanded w_v: wvdiag[h, q, c] = w_v[h] * (q == c)
            wvdiag = constp.tile([P, P, P], BF16)
            nc.sync.dma_start(wvdiag[:], wvdiag_ext[:, :, :])

            pt_sb = constp.tile([P, KC, Q], BF16)  # P^T chunks [k, (kc, q)]

            G = 8  # queries per grouped tanh instruction
            for qb in range(QB):
                sc_ps = psp.tile([P, KH, 512], FP32, tag="sc")
                for g in range(P // G):
                    # VectorE: pre-add B^T + A^T[:,q] for G queries (bf16 4x mode)
                    s_t = featp.tile([H, G, K], BF16, tag="s")
                    for j in range(G):
                        qg = qb * P + g * G + j
                        nc.vector.tensor_scalar_add(
                            s_t[:, j, :], bt_sb[:], at_sb[:, qg : qg + 1]
                        )
                    # ScalarE: one big tanh over the whole group (FD = G*K)
                    ft = featp.tile([H, G, K], BF16, tag="ft")
                    nc.scalar.activation(ft[:], s_t[:], ACT.Tanh)
                    # TensorE: accumulate score rows into PSUM partitions
                    for j in range(G):
                        qi = g * G + j
                        for kh in range(KH):
                            nc.tensor.matmul(
                                sc_ps[:, kh, :], wvdiag[:, qi, :],
                                ft[:, j, kh * 512 : (kh + 1) * 512],
                                start=(qi == 0), stop=(qi == P - 1),
                                skip_group_check=True,
                            )
                    if g == 0:
                        # additive -1e6 mask on every row: ones^T (x) mask_row.
                        # Emitted early (accumulation is commutative) so the
                        # epilogue exp doesn't wait on it.
                        for kh in range(KH):
                            nc.tensor.matmul(
                                sc_ps[:, kh, :], ones_sb[:],
                                mrow_sb[:, kh * 512 : (kh + 1) * 512],
                                start=False, stop=False, skip_group_check=True,
                            )
                # softmax (no rowmax needed: |scores| <= sum|w_v| ~ 11)
                e_sb = sbp.tile([P, KH, 512], BF16, tag="e")
                nc.scalar.activation(e_sb[:], sc_ps[:], ACT.Exp)
                z_sb = sbp.tile([P, 1], FP32, tag="z")
                nc.vector.tensor_reduce(
                    z_sb[:], e_sb[:], axis=mybir.AxisListType.XY, op=mybir.AluOpType.add
                )
                r_sb = sbp.tile([P, 1], FP32, tag="r")
                nc.vector.reciprocal(r_sb[:], z_sb[:])
                pw_sb = sbp.tile([P, KH, 512], BF16, tag="pw")
                nc.vector.tensor_scalar_mul(pw_sb[:], e_sb[:], r_sb[:])
                # transpose P -> P^T chunks
                for kc in range(KC):
                    tp_ps = psp2.tile([P, P], BF16, tag="tp")
                    nc.tensor.transpose(
                        tp_ps[:],
                        pw_sb[:, kc // 4, (kc % 4) * P : (kc % 4 + 1) * P],
                        ident[:],
                    )
                    nc.vector.tensor_copy(pt_sb[:, kc, qb * P : (qb + 1) * P], tp_ps[:])
                # out[q, v] = sum_k P^T[k, q] * V[k, v]
                o_ps = psp2.tile([P, DV], FP32, tag="tp")
                for kc in range(KC):
                    nc.tensor.matmul(
                        o_ps[:], pt_sb[:, kc, qb * P : (qb + 1) * P], v_sb[:, kc, :],
                        start=(kc == 0), stop=(kc == KC - 1),
                    )
                o_sb = sbp.tile([P, DV], FP32, tag="o")
                nc.vector.tensor_copy(o_sb[:], o_ps[:])
                nc.sync.dma_start(out_ext[qb * P : (qb + 1) * P, :], o_sb[:])

    nc.compile()
    return nc


def _get_nc():
    global _NC
    if _NC is None:
        _NC = _build()
    return _NC


def kernel(queries, keys, values, valid_lens, W_q, W_k, w_v):
    global LAST_RESULT
    queries = np.asarray(queries, dtype=np.float32)
    keys = np.asarray(keys, dtype=np.float32)
    values = np.asarray(values, dtype=np.float32)
    valid_lens = np.asarray(valid_lens, dtype=np.int32)
    W_q = np.asarray(W_q, dtype=np.float32)
    W_k = np.asarray(W_k, dtype=np.float32)
    w_v = np.asarray(w_v, dtype=np.float32)

    wqT = np.ascontiguousarray(W_q.T).astype(BF16_NP)       # [DQ, H]
    wkT = np.ascontiguousarray(W_k.T).astype(BF16_NP)       # [DK, H]
    wvdiag = np.zeros((H, P, P), dtype=np.float32)
    wvdiag[:, np.arange(P), np.arange(P)] = w_v[:, None]
    wvdiag = wvdiag.astype(BF16_NP)
    ar = np.arange(K)

    in_maps = []
    for b in range(B):
        mrow = np.where(ar < int(valid_lens[b]), 0.0, NEG).astype(np.float32)
        in_maps.append({
            "qT": np.ascontiguousarray(queries[b].T).astype(BF16_NP),
            "kT": np.ascontiguousarray(keys[b].T).astype(BF16_NP),
            "v": np.ascontiguousarray(values[b]).astype(BF16_NP),
            "wqT": wqT,
            "wkT": wkT,
            "wvdiag": wvdiag,
            "mrow": mrow[None, :].astype(BF16_NP),
            "out": np.zeros((Q, DV), dtype=np.float32),
        })

    nc = _get_nc()
    trace = bool(int(os.environ.get("KERNEL_TRACE", "0")))
    res = run_bass_kernel_spmd(nc, in_maps, core_ids=list(range(B)), trace=trace)
    LAST_RESULT = res
    out = np.stack([np.asarray(res.results[i]["out"], dtype=np.float32) for i in range(B)])
    return out


# revision 24
# speedup vs baseline: 1.0006x; 1.0006x over previous
"""Trainium2 Bass kernel for AdditiveAttention (nn_AdditiveAttention_44564580663638).

Data-parallel over batch: B=8 -> one batch element per NeuronCore (8 cores).

Per-core pipeline (Q=256, K=1024, D=256, H=128, DV=128):
  1. TensorE: project  A^T = W_q @ q^T  [H,Q],  B^T = W_k @ k^T  [H,K]
  2. ScalarE (the bottleneck): per query q, one activation instruction computes
     feat = tanh(B^T + A^T[:,q])  [128h x 1024k]  (broadcast-add fused via bias)
  3. TensorE: scores accumulate via one-hot-expanded w_v weights:
     lhsT_q[h,c] = w_v[h] * (c==q)  => matmul writes w_v . feat into PSUM
     partition q; 128 accumulating matmuls build the [128q, 512k] score block.
     The -1e6 softmax mask folds in as one extra K=1 matmul (ones x mask_row).
  4. ScalarE: E = exp(scores) straight out of PSUM; VectorE: Z, 1/Z, P = E/Z
  5. TensorE: transpose P -> P^T chunks; final matmul P^T.T @ V -> out [256,128]
"""

import os
import sys

for _p in ("/opt/trn_rl_repo", "/root/.axon_site/_ro/trn_rl_repo"):
    if os.path.isdir(_p) and _p not in sys.path:
        sys.path.insert(0, _p)

import numpy as np
import ml_dtypes

import concourse.bass as bass
import concourse.bacc as bacc
import concourse.tile as tile
import concourse.mybir as mybir
from concourse.bass_utils import run_bass_kernel_spmd
from concourse.masks import make_identity

B, Q, K, DQ, DK, H, DV = 8, 256, 1024, 256, 256, 128, 128
P = 128
QB = Q // P      # query blocks of 128
KC = K // P      # key chunks of 128
DC = DQ // P     # contraction chunks for the projections
KH = K // 512    # psum half-banks per score row
FP32 = mybir.dt.float32
BF16 = mybir.dt.bfloat16
BF16_NP = ml_dtypes.bfloat16
NEG = -1e6

_NC = None
LAST_RESULT = None


def _build():
    nc = bacc.Bacc("TRN2", target_bir_lowering=False, debug=False)
    ACT = mybir.ActivationFunctionType

    qT_ext = nc.declare_dram_parameter("qT", [DQ, Q], BF16, isOutput=False)
    kT_ext = nc.declare_dram_parameter("kT", [DK, K], BF16, isOutput=False)
    v_ext = nc.declare_dram_parameter("v", [K, DV], BF16, isOutput=False)
    wqT_ext = nc.declare_dram_parameter("wqT", [DQ, H], BF16, isOutput=False)
    wkT_ext = nc.declare_dram_parameter("wkT", [DK, H], BF16, isOutput=False)
    wv_ext = nc.declare_dram_parameter("wv", [H, 1], BF16, isOutput=False)
    mrow_ext = nc.declare_dram_parameter("mrow", [1, K], BF16, isOutput=False)
    out_ext = nc.declare_dram_parameter("out", [Q, DV], FP32, isOutput=True)

    with tile.TileContext(nc) as tc:
        with (
            tc.tile_pool(name="const", bufs=1) as constp,
            tc.tile_pool(name="sb", bufs=2) as sbp,
            tc.tile_pool(name="feat", bufs=3) as featp,
            tc.tile_pool(name="ps", bufs=2, space="PSUM") as psp,
            tc.tile_pool(name="ps2", bufs=2, space="PSUM") as psp2,
        ):
            # critical-path inputs first: kT/wk (for B^T), qT/wq (for A^T)
            kT_sb = constp.tile([P, DC, K], BF16)
            for c in range(DC):
                for s in range(2):
                    nc.sync.dma_start(
                        kT_sb[:, c, s * 512 : (s + 1) * 512],
                        kT_ext[c * P : (c + 1) * P, s * 512 : (s + 1) * 512],
                    )
            wk_sb = constp.tile([P, DC, H], BF16)
            nc.sync.dma_start(wk_sb[:], wkT_ext.rearrange("(c p) h -> p c h", p=P))
            qT_sb = constp.tile([P, DC, Q], BF16)
            for c in range(DC):
                nc.sync.dma_start(qT_sb[:, c, :], qT_ext[c * P : (c + 1) * P, :])
            wq_sb = constp.tile([P, DC, H], BF16)
            nc.sync.dma_start(wq_sb[:], wqT_ext.rearrange("(c p) h -> p c h", p=P))

            # projections (bf16 results: feed the DVE 4x-mode pre-adds)
            at_sb = constp.tile([H, Q], FP32)
            at_ps = psp.tile([H, Q], FP32, tag="sc")
            for c in range(DC):
                nc.tensor.matmul(
                    at_ps[:], wq_sb[:, c, :], qT_sb[:, c, :],
                    start=(c == 0), stop=(c == DC - 1),
                )
            nc.vector.tensor_copy(at_sb[:], at_ps[:])

            bt_sb = constp.tile([H, K], BF16)
            for kh in range(KH):
                bt_ps = psp.tile([H, 512], FP32, tag="sc")
                for c in range(DC):
                    nc.tensor.matmul(
                        bt_ps[:], wk_sb[:, c, :], kT_sb[:, c, kh * 512 : (kh + 1) * 512],
                        start=(c == 0), stop=(c == DC - 1),
                    )
                nc.vector.tensor_copy(bt_sb[:, kh * 512 : (kh + 1) * 512], bt_ps[:])

            # non-critical inputs after the projection chain
            mrow_sb = constp.tile([1, K], BF16)
            nc.sync.dma_start(mrow_sb[:], mrow_ext[:, :])
            v_sb = constp.tile([P, KC, DV], BF16)
            nc.sync.dma_start(v_sb[:], v_ext.rearrange("(c p) d -> p c d", p=P))
            ones_sb = constp.tile([1, P], BF16)
            nc.vector.memset(ones_sb[:], 1.0)
            ident = constp.tile([P, P], BF16)
            make_identity(nc, ident[:])
            # sliding-window one-hot w_v: wvstrip[:, P:P+1] = w_v, zeros
            # elsewhere; wvstrip[:, P-qi : 2P-qi] is then w_v (x) e_qi^T.
            wv_sb = constp.tile([H, 1], BF16)
            nc.sync.dma_start(wv_sb[:], wv_ext[:, :])
            wvstrip = constp.tile([H, 2 * P + 1], BF16)
            nc.vector.memset(wvstrip[:], 0.0)
            nc.vector.tensor_copy(wvstrip[:, P : P + 1], wv_sb[:, :])

            pt_sb = constp.tile([P, KC, Q], BF16)  # P^T chunks [k, (kc, q)]

            G = 8  # queries per grouped tanh instruction
            for qb in range(QB):
                sc_ps = psp.tile([P, KH, 512], FP32, tag="sc")
                for g in range(P // G):
                    # VectorE: pre-add B^T + A^T[:,q] for G queries (bf16 4x mode)
                    s_t = featp.tile([H, G, K], BF16, tag="s")
                    for j in range(G):
                        qg = qb * P + g * G + j
                        nc.vector.tensor_scalar_add(
                            s_t[:, j, :], bt_sb[:], at_sb[:, qg : qg + 1]
                        )
                    # ScalarE: one big tanh over the whole group (FD = G*K)
                    ft = featp.tile([H, G, K], BF16, tag="ft")
                    nc.scalar.activation(ft[:], s_t[:], ACT.Tanh)
                    # TensorE: accumulate score rows into PSUM partitions
                    for j in range(G):
                        qi = g * G + j
                        for kh in range(KH):
                            nc.tensor.matmul(
                                sc_ps[:, kh, :], wvstrip[:, P - qi : 2 * P - qi],
                                ft[:, j, kh * 512 : (kh + 1) * 512],
                                start=(qi == 0), stop=(qi == P - 1),
                                skip_group_check=True,
                            )
                    if g == 0:
                        # additive -1e6 mask on every row: ones^T (x) mask_row.
                        # Emitted early (accumulation is commutative) so the
                        # epilogue exp doesn't wait on it.
                        for kh in range(KH):
                            nc.tensor.matmul(
                                sc_ps[:, kh, :], ones_sb[:],
                                mrow_sb[:, kh * 512 : (kh + 1) * 512],
                                start=False, stop=False, skip_group_check=True,
                            )
                # softmax (no rowmax needed: |scores| <= sum|w_v| ~ 11)
                e_sb = sbp.tile([P, KH, 512], BF16, tag="e")
                nc.scalar.activation(e_sb[:], sc_ps[:], ACT.Exp)
                z_sb = sbp.tile([P, 1], FP32, tag="z")
                nc.vector.tensor_reduce(
                    z_sb[:], e_sb[:], axis=mybir.AxisListType.XY, op=mybir.AluOpType.add
                )
                r_sb = sbp.tile([P, 1], FP32, tag="r")
                nc.vector.reciprocal(r_sb[:], z_sb[:])
                pw_sb = sbp.tile([P, KH, 512], BF16, tag="pw")
                nc.vector.tensor_scalar_mul(pw_sb[:], e_sb[:], r_sb[:])
                # transpose P -> P^T chunks
                for kc in range(KC):
                    tp_ps = psp2.tile([P, P], BF16, tag="tp")
                    nc.tensor.transpose(
                        tp_ps[:],
                        pw_sb[:, kc // 4, (kc % 4) * P : (kc % 4 + 1) * P],
                        ident[:],
                    )
                    nc.vector.tensor_copy(pt_sb[:, kc, qb * P : (qb + 1) * P], tp_ps[:])
                # out[q, v] = sum_k P^T[k, q] * V[k, v]
                o_ps = psp2.tile([P, DV], FP32, tag="tp")
                for kc in range(KC):
                    nc.tensor.matmul(
                        o_ps[:], pt_sb[:, kc, qb * P : (qb + 1) * P], v_sb[:, kc, :],
                        start=(kc == 0), stop=(kc == KC - 1),
                    )
                o_sb = sbp.tile([P, DV], FP32, tag="o")
                nc.vector.tensor_copy(o_sb[:], o_ps[:])
                nc.sync.dma_start(out_ext[qb * P : (qb + 1) * P, :], o_sb[:])

    nc.compile()
    return nc


def _get_nc():
    global _NC
    if _NC is None:
        _NC = _build()
    return _NC


def kernel(queries, keys, values, valid_lens, W_q, W_k, w_v):
    global LAST_RESULT
    queries = np.asarray(queries, dtype=np.float32)
    keys = np.asarray(keys, dtype=np.float32)
    values = np.asarray(values, dtype=np.float32)
    valid_lens = np.asarray(valid_lens, dtype=np.int32)
    W_q = np.asarray(W_q, dtype=np.float32)
    W_k = np.asarray(W_k, dtype=np.float32)
    w_v = np.asarray(w_v, dtype=np.float32)

    wqT = np.ascontiguousarray(W_q.T).astype(BF16_NP)       # [DQ, H]
    wkT = np.ascontiguousarray(W_k.T).astype(BF16_NP)       # [DK, H]
    wvc = np.ascontiguousarray(w_v[:, None]).astype(BF16_NP)  # [H, 1]
    ar = np.arange(K)

    in_maps = []
    for b in range(B):
        mrow = np.where(ar < int(valid_lens[b]), 0.0, NEG).astype(np.float32)
        in_maps.append({
            "qT": np.ascontiguousarray(queries[b].T).astype(BF16_NP),
            "kT": np.ascontiguousarray(keys[b].T).astype(BF16_NP),
            "v": np.ascontiguousarray(values[b]).astype(BF16_NP),
            "wqT": wqT,
            "wkT": wkT,
            "wv": wvc,
            "mrow": mrow[None, :].astype(BF16_NP),
            "out": np.zeros((Q, DV), dtype=np.float32),
        })

    nc = _get_nc()
    trace = bool(int(os.environ.get("KERNEL_TRACE", "0")))
    res = run_bass_kernel_spmd(nc, in_maps, core_ids=list(range(B)), trace=trace)
    LAST_RESULT = res
    out = np.stack([np.asarray(res.results[i]["out"], dtype=np.float32) for i in range(B)])
    return out
